# revision 14
# baseline (speedup 1.0000x reference)
"""Trainium2 Bass kernel for nn_Loss_9749575762182.

Computes two scalar losses over (8192, 2048) fp32 tensors:
  wmse = mean((weight[:,None] * (target - input))**2)
  wcl  = mean(|(st*ln(tp+eps) + (1-st)*ln(1-tp+eps)) * obrT|)

Strategy: data-parallel over the row axis across 8 NeuronCores
(1024 rows each), eight [128, 2048] tiles per core streamed through
SBUF. The tiny per-partition partial sums land back in DRAM and the
host finishes the reduction in float64.

v2 restructure (vs the 152us baseline): the baseline serialized on a
per-tile cross-engine chain Ln2 -> sub -> mul -> add -> mul -> Abs
(~18.7us/tile, engines only ~60% busy). Key identity: obrT >= 0,
st in [0,1], and both logs are <= 0, so
  |bce*ob| = -bce*ob = |u*l1| + |v*l2|,  u = st*ob, v = (1-st)*ob
which splits into two INDEPENDENT depth-1 chains after each Ln.
u and v' = (st-1)*ob depend only on DMA'd inputs (v' via one
scalar_tensor_tensor op), and the products are accumulated directly
on DVE via scalar_tensor_tensor(accum_out=...), removing the ACT Abs
pass entirely:
  ACT: l1 = Ln(tp + eps); l2 = Ln(-tp + (1+eps)); Square(w*diff)+accum
  DVE: diff = tgt - in; u = st*ob; v' = (st-1)*ob [STT];
       a = l1*u  +accum (<= 0);  b = l2*v' +accum (>= 0)
  wcl_sum = -sum(a) + sum(b)   (signs resolved on the host)

Hard-won environment notes (axon-tunneled trn2, this toolchain):
  - Build on bacc.Bacc() and call nc.finalize() before run_bass_via_pjrt;
    raw bass.Bass() BIR fails walrus ("Reg has not been allocated"), and
    without Bacc's generate_event_semaphores pass any instruction with
    >1 semaphore wait dies in codegen ("Too many sync wait commands").
  - tensor_tensor_reduce compiles + simulates fine but faults on real HW
    via the PJRT path; scalar_tensor_tensor with accum_out is the DVE
    accumulation that does work (ACT activation accum_out also works).
  - Big loads go through nc.sync.dma_start (HW-DGE, fans out across HW
    queues): all-gpsimd SWDGE funnels through ONE dynamic queue
    (~216 GB/s ceiling observed -> 185us); HW-DGE gets 153us.
"""

import os
import sys

if "/opt/trn_rl_repo" not in sys.path:
    sys.path.insert(0, "/opt/trn_rl_repo")

import numpy as np

N, D = 8192, 2048
NCORES = 8
ROWS = N // NCORES  # rows per core
P = 128             # SBUF partitions
EPS = 1e-10

_CACHE = {}


def build(rows=ROWS, d=D, bufs=3):
    import concourse.bacc as bacc
    import concourse.tile as tile
    from concourse import mybir

    f32 = mybir.dt.float32
    ALU = mybir.AluOpType
    ACTF = mybir.ActivationFunctionType
    nt = rows // P
    ca = nt + 1  # accum columns per cl term (last tile split in halves)

    nc = bacc.Bacc()
    inp = nc.dram_tensor("input", [rows, d], f32, kind="ExternalInput")
    tgt = nc.dram_tensor("target", [rows, d], f32, kind="ExternalInput")
    wgt = nc.dram_tensor("weight", [rows], f32, kind="ExternalInput")
    st = nc.dram_tensor("sub_target", [rows, d], f32, kind="ExternalInput")
    tp = nc.dram_tensor("target_pre", [rows, d], f32, kind="ExternalInput")
    ob = nc.dram_tensor("sub_obrT", [rows, d], f32, kind="ExternalInput")
    # mse partials (ACT-written); cl partials (DVE-written): a-term in
    # cols [0, ca), b-term in cols [ca, 2*ca)
    out_mse = nc.dram_tensor("mse_partials", [P, nt], f32, kind="ExternalOutput")
    out_cl = nc.dram_tensor("cl_partials", [P, 2 * ca], f32, kind="ExternalOutput")

    inp_t = inp.rearrange("(t p) d -> t p d", p=P)
    tgt_t = tgt.rearrange("(t p) d -> t p d", p=P)
    st_t = st.rearrange("(t p) d -> t p d", p=P)
    tp_t = tp.rearrange("(t p) d -> t p d", p=P)
    ob_t = ob.rearrange("(t p) d -> t p d", p=P)
    wgt_t = wgt.rearrange("(t p) -> p t", p=P)

    with tile.TileContext(nc) as tc:
        with (
            tc.tile_pool(name="singles", bufs=1) as singles,
            tc.tile_pool(name="in_p", bufs=bufs) as in_p,
            tc.tile_pool(name="tgt_p", bufs=bufs) as tgt_p,
            tc.tile_pool(name="st_p", bufs=bufs) as st_p,
            tc.tile_pool(name="tp_p", bufs=bufs) as tp_p,
            tc.tile_pool(name="ob_p", bufs=bufs) as ob_p,
            tc.tile_pool(name="l1_p", bufs=2) as l1_p,
            tc.tile_pool(name="l2_p", bufs=2) as l2_p,
            tc.tile_pool(name="u_p", bufs=2) as u_p,
            tc.tile_pool(name="v_p", bufs=2) as v_p,
        ):
            # w_cols via SWDGE: keeps the tiny strided load (and its many
            # small descriptors) off the head of the qSP HWDGE FIFO that
            # streams the 40 big tile loads.
            w_cols = singles.tile([P, nt], f32)
            nc.gpsimd.dma_start(out=w_cols, in_=wgt_t)
            mse_cols = singles.tile([P, nt], f32)
            cl_cols = singles.tile([P, 2 * ca], f32)
            eps_b = singles.tile([P, 1], f32)
            nc.vector.memset(eps_b, EPS)
            one_eps_b = singles.tile([P, 1], f32)
            nc.vector.memset(one_eps_b, 1.0 + EPS)

            # Each instruction should depend on at most ONE foreign
            # semaphore that is not yet known-satisfied; tiny "touch" ops
            # consume extra waits so the real consumers inherit them via
            # engine program order / already-observed clocks.
            touch_d = singles.tile([P, 1], f32)
            atouch_d = singles.tile([P, 1], f32)
            nc.scalar.activation(
                out=atouch_d, in_=w_cols[:, 0:1], func=ACTF.Copy
            )  # waits w_cols DMA on ACT
            nc.scalar.activation(
                out=atouch_d, in_=eps_b, func=ACTF.Copy
            )  # waits DVE memsets on ACT
            nc.scalar.activation(out=atouch_d, in_=one_eps_b, func=ACTF.Copy)

            # Pipeline shape (steady state, DMA-paced at ~12.7us/tile):
            #   DMA batch order g,x,s,o,q matches DVE op order, so DVE is
            #   never input-starved. The a/b accumulation ops LAG ONE TILE:
            #   a(t-1)/b(t-1) consume Ln outputs finished a full tile ago,
            #   so no DVE op ever waits on an ACT op of its own tile -- the
            #   cross-engine chains that serialized earlier versions
            #   (~15.5us/tile) are gone. DVE (~11.3us/tile) rides just
            #   behind the free-running 414 GB/s DMA stream.
            # The LAST tile's q is loaded in two halves and its Ln/accum
            # chain runs half-width, shrinking the post-stream drain from
            # ~6.5us to ~4.5us.
            d2 = d // 2
            prev = None
            for t in range(nt):
                # Two independent DMA paths so neither ring's credit pacing
                # caps the stream: s,o (2MB/tile) go via the otherwise-idle
                # SWDGE queue; g,x,q (3MB/tile) via the qSP HWDGE ring.
                g = tgt_p.tile([P, d], f32)
                nc.sync.dma_start(out=g, in_=tgt_t[t])
                x = in_p.tile([P, d], f32)
                nc.sync.dma_start(out=x, in_=inp_t[t])
                s = st_p.tile([P, d], f32)
                nc.gpsimd.dma_start(out=s, in_=st_t[t])
                o = ob_p.tile([P, d], f32)
                nc.gpsimd.dma_start(out=o, in_=ob_t[t])
                q = tp_p.tile([P, d], f32)
                if t < nt - 1:
                    nc.sync.dma_start(out=q, in_=tp_t[t])
                else:
                    nc.sync.dma_start(out=q[:, 0:d2], in_=tp_t[t][:, 0:d2])
                    nc.sync.dma_start(out=q[:, d2:d], in_=tp_t[t][:, d2:d])

                # ---- wmse: diff on DVE, then Square(w*diff)+accum on ACT
                nc.vector.tensor_copy(touch_d, g[:, 0:1])  # consume g-DMA wait
                nc.vector.tensor_sub(g, g, x)  # g <- diff = target - input

                # ---- lagged accumulation of the previous tile's products
                if prev is not None:
                    pl1, pl2, pu, pv, pt = prev
                    nc.vector.scalar_tensor_tensor(
                        out=pl1,  # sink, in place
                        in0=pl1,
                        scalar=0.0,
                        in1=pu,
                        op0=ALU.bypass,
                        op1=ALU.mult,
                        accum_out=cl_cols[:, pt : pt + 1],
                    )
                    nc.vector.scalar_tensor_tensor(
                        out=pl2,  # sink, in place
                        in0=pl2,
                        scalar=0.0,
                        in1=pv,
                        op0=ALU.bypass,
                        op1=ALU.mult,
                        accum_out=cl_cols[:, ca + pt : ca + pt + 1],
                    )

                # ---- wcl inputs: u = st*ob; v' = (st-1)*ob (both pre-Ln)
                nc.vector.tensor_copy(touch_d, s[:, 0:1])  # consume s-DMA wait
                u = u_p.tile([P, d], f32)
                nc.vector.tensor_mul(u, s, o)
                v = v_p.tile([P, d], f32)
                nc.vector.scalar_tensor_tensor(
                    out=v,  # v <- v' = (st - 1) * ob
                    in0=s,
                    scalar=1.0,
                    in1=o,
                    op0=ALU.subtract,
                    op1=ALU.mult,
                )

                # ---- logs (ACT); bias/scale fold the affine into the LUT
                l1 = l1_p.tile([P, d], f32)
                l2 = l2_p.tile([P, d], f32)
                if t < nt - 1:
                    nc.scalar.activation(
                        out=l1, in_=q, func=ACTF.Ln, bias=eps_b, scale=1.0
                    )
                    nc.scalar.activation(
                        out=l2, in_=q, func=ACTF.Ln, bias=one_eps_b, scale=-1.0
                    )
                else:
                    for h0, h1 in ((0, d2), (d2, d)):
                        nc.scalar.activation(
                            out=l1[:, h0:h1],
                            in_=q[:, h0:h1],
                            func=ACTF.Ln,
                            bias=eps_b,
                            scale=1.0,
                        )
                        nc.scalar.activation(
                            out=l2[:, h0:h1],
                            in_=q[:, h0:h1],
                            func=ACTF.Ln,
                            bias=one_eps_b,
                            scale=-1.0,
                        )
                # Square AFTER the Lns in ACT program order: it feeds only
                # the mse store, so it must not sit between q landing and
                # the Ln->a/b tail chain.
                nc.scalar.activation(
                    out=x,  # sink; x is dead after the sub
                    in_=g,
                    func=ACTF.Square,
                    bias=0.0,
                    scale=w_cols[:, t : t + 1],
                    accum_out=mse_cols[:, t : t + 1],
                )
                prev = (l1, l2, u, v, t)

            # ---- final tile's lagged accums, half-width to shorten drain
            pl1, pl2, pu, pv, pt = prev
            for i, (h0, h1) in enumerate(((0, d2), (d2, d))):
                nc.vector.scalar_tensor_tensor(
                    out=pl1[:, h0:h1],
                    in0=pl1[:, h0:h1],
                    scalar=0.0,
                    in1=pu[:, h0:h1],
                    op0=ALU.bypass,
                    op1=ALU.mult,
                    accum_out=cl_cols[:, pt + i : pt + i + 1],
                )
                nc.vector.scalar_tensor_tensor(
                    out=pl2[:, h0:h1],
                    in0=pl2[:, h0:h1],
                    scalar=0.0,
                    in1=pv[:, h0:h1],
                    op0=ALU.bypass,
                    op1=ALU.mult,
                    accum_out=cl_cols[:, ca + pt + i : ca + pt + i + 1],
                )

            nc.sync.dma_start(out=out_mse[:, :], in_=mse_cols)
            nc.sync.dma_start(out=out_cl[:, :], in_=cl_cols)
    return nc


def _get_nc():
    bufs = int(os.environ.get("BASS_BUFS", "3"))
    if bufs not in _CACHE:
        nc = build(bufs=bufs)
        nc.finalize()  # runs Bacc's passes (event-sem wait splitting, regalloc)
        _CACHE[bufs] = nc
    return _CACHE[bufs]


def _install_profile_hook():
    """Register the NTFF profile hook that this container's stripped antenv
    lacks: a ctypes bridge into libaxon_pjrt.so (same ABI trn_boot.py uses).
    Only needed for trace=True runs."""
    if "antenv.axon_hooks" in sys.modules:
        return
    import contextlib
    import ctypes
    import types

    so_path = "/opt/axon/libaxon_pjrt.so"
    lib = ctypes.CDLL(so_path)
    if not hasattr(lib, "axon_start_nrt_profile"):
        return
    lib.axon_start_nrt_profile.argtypes = [
        ctypes.POINTER(ctypes.c_int64),
        ctypes.c_size_t,
    ]
    lib.axon_start_nrt_profile.restype = ctypes.c_int64
    lib.axon_stop_nrt_profile.argtypes = [ctypes.c_char_p]
    lib.axon_stop_nrt_profile.restype = ctypes.c_int64

    @contextlib.contextmanager
    def _hook(output_dir, device_ids):
        import jax

        jax.devices()
        if device_ids:
            ids = (ctypes.c_int64 * len(device_ids))(*device_ids)
            rc = lib.axon_start_nrt_profile(ids, len(device_ids))
        else:
            rc = lib.axon_start_nrt_profile(None, 0)
        if rc != 0:
            raise RuntimeError(f"axon_start_nrt_profile rc={rc}")
        try:
            yield
        finally:
            n = lib.axon_stop_nrt_profile(str(output_dir).encode())
            print(f"profile: {n} file(s) written to {output_dir}")

    mod = types.ModuleType("antenv.axon_hooks")
    mod.get_axon_ntff_profile_hook = lambda: _hook
    sys.modules["antenv.axon_hooks"] = mod


def kernel(**inputs):
    from concourse.bass_utils import run_bass_kernel_spmd

    nc = _get_nc()
    names = ["input", "target", "weight", "sub_target", "target_pre", "sub_obrT"]
    arrs = {k: np.ascontiguousarray(np.asarray(inputs[k], dtype=np.float32)) for k in names}
    in_maps = []
    for c in range(NCORES):
        sl = slice(c * ROWS, (c + 1) * ROWS)
        in_maps.append({k: np.ascontiguousarray(v[sl]) for k, v in arrs.items()})

    trace = os.environ.get("BASS_KERNEL_PROFILE", "0") == "1"
    if trace:
        _install_profile_hook()
    res = run_bass_kernel_spmd(nc, in_maps, list(range(NCORES)), trace=trace)

    ca = ROWS // P + 1
    mse_sum = 0.0
    cla_sum = 0.0
    clb_sum = 0.0
    for r in res.results:
        mse_sum += np.asarray(r["mse_partials"], dtype=np.float64).sum()
        cl = np.asarray(r["cl_partials"], dtype=np.float64)
        cla_sum += cl[:, :ca].sum()
        clb_sum += cl[:, ca:].sum()
    tot = float(N) * float(D)
    if trace and res.exec_time_ns is not None:
        print(f"HW exec time: {res.exec_time_ns} ns")
    return (
        np.asarray(np.float32(mse_sum / tot)),
        np.asarray(np.float32((clb_sum - cla_sum) / tot)),
    )


# revision 15
# speedup vs baseline: 1.3668x; 1.3668x over previous
"""Trainium2 Bass kernel for nn_Loss_9749575762182.

Computes two scalar losses over (8192, 2048) fp32 tensors:
  wmse = mean((weight[:,None] * (target - input))**2)
  wcl  = mean(|(st*ln(tp+eps) + (1-st)*ln(1-tp+eps)) * obrT|)

Strategy: data-parallel over the row axis across 8 NeuronCores
(1024 rows each), eight [128, 2048] tiles per core streamed through
SBUF. The tiny per-partition partial sums land back in DRAM and the
host finishes the reduction in float64.

v2 restructure (vs the 152us baseline): the baseline serialized on a
per-tile cross-engine chain Ln2 -> sub -> mul -> add -> mul -> Abs
(~18.7us/tile, engines only ~60% busy). Key identity: obrT >= 0,
st in [0,1], and both logs are <= 0, so
  |bce*ob| = -bce*ob = |u*l1| + |v*l2|,  u = st*ob, v = (1-st)*ob
which splits into two INDEPENDENT depth-1 chains after each Ln.
u and v' = (st-1)*ob depend only on DMA'd inputs (v' via one
scalar_tensor_tensor op), and the products are accumulated directly
on DVE via scalar_tensor_tensor(accum_out=...), removing the ACT Abs
pass entirely:
  ACT: l1 = Ln(tp + eps); l2 = Ln(-tp + (1+eps)); Square(w*diff)+accum
  DVE: diff = tgt - in; u = st*ob; v' = (st-1)*ob [STT];
       a = l1*u  +accum (<= 0);  b = l2*v' +accum (>= 0)
  wcl_sum = -sum(a) + sum(b)   (signs resolved on the host)

Hard-won environment notes (axon-tunneled trn2, this toolchain):
  - Build on bacc.Bacc() and call nc.finalize() before run_bass_via_pjrt;
    raw bass.Bass() BIR fails walrus ("Reg has not been allocated"), and
    without Bacc's generate_event_semaphores pass any instruction with
    >1 semaphore wait dies in codegen ("Too many sync wait commands").
  - tensor_tensor_reduce compiles + simulates fine but faults on real HW
    via the PJRT path; scalar_tensor_tensor with accum_out is the DVE
    accumulation that does work (ACT activation accum_out also works).
  - Big loads go through nc.sync.dma_start (HW-DGE, fans out across HW
    queues): all-gpsimd SWDGE funnels through ONE dynamic queue
    (~216 GB/s ceiling observed -> 185us); HW-DGE gets 153us.
"""

import os
import sys

if "/opt/trn_rl_repo" not in sys.path:
    sys.path.insert(0, "/opt/trn_rl_repo")

import numpy as np

N, D = 8192, 2048
NCORES = 8
ROWS = N // NCORES  # rows per core
P = 128             # SBUF partitions
EPS = 1e-10

_CACHE = {}


def build(rows=ROWS, d=D, bufs=3):
    import concourse.bacc as bacc
    import concourse.tile as tile
    from concourse import mybir

    f32 = mybir.dt.float32
    ALU = mybir.AluOpType
    ACTF = mybir.ActivationFunctionType
    nt = rows // P
    ca = nt + 1  # accum columns per cl term (last tile split in halves)

    nc = bacc.Bacc()
    inp = nc.dram_tensor("input", [rows, d], f32, kind="ExternalInput")
    tgt = nc.dram_tensor("target", [rows, d], f32, kind="ExternalInput")
    wgt = nc.dram_tensor("weight", [rows], f32, kind="ExternalInput")
    st = nc.dram_tensor("sub_target", [rows, d], f32, kind="ExternalInput")
    tp = nc.dram_tensor("target_pre", [rows, d], f32, kind="ExternalInput")
    ob = nc.dram_tensor("sub_obrT", [rows, d], f32, kind="ExternalInput")
    # mse partials (ACT-written); cl partials (DVE-written): a-term in
    # cols [0, ca), b-term in cols [ca, 2*ca)
    out_mse = nc.dram_tensor("mse_partials", [P, nt], f32, kind="ExternalOutput")
    out_cl = nc.dram_tensor("cl_partials", [P, 2 * ca], f32, kind="ExternalOutput")

    inp_t = inp.rearrange("(t p) d -> t p d", p=P)
    tgt_t = tgt.rearrange("(t p) d -> t p d", p=P)
    st_t = st.rearrange("(t p) d -> t p d", p=P)
    tp_t = tp.rearrange("(t p) d -> t p d", p=P)
    ob_t = ob.rearrange("(t p) d -> t p d", p=P)
    wgt_t = wgt.rearrange("(t p) -> p t", p=P)

    with tile.TileContext(nc) as tc:
        with (
            tc.tile_pool(name="singles", bufs=1) as singles,
            tc.tile_pool(name="in_p", bufs=bufs) as in_p,
            tc.tile_pool(name="tgt_p", bufs=bufs) as tgt_p,
            tc.tile_pool(name="st_p", bufs=bufs) as st_p,
            tc.tile_pool(name="tp_p", bufs=bufs) as tp_p,
            tc.tile_pool(name="ob_p", bufs=bufs) as ob_p,
            tc.tile_pool(name="l1_p", bufs=2) as l1_p,
            tc.tile_pool(name="l2_p", bufs=2) as l2_p,
            tc.tile_pool(name="u_p", bufs=2) as u_p,
            tc.tile_pool(name="v_p", bufs=2) as v_p,
        ):
            # w_cols via SWDGE: keeps the tiny strided load (and its many
            # small descriptors) off the head of the qSP HWDGE FIFO that
            # streams the 40 big tile loads.
            w_cols = singles.tile([P, nt], f32)
            nc.gpsimd.dma_start(out=w_cols, in_=wgt_t)
            mse_cols = singles.tile([P, nt], f32)
            cl_cols = singles.tile([P, 2 * ca], f32)
            eps_b = singles.tile([P, 1], f32)
            nc.vector.memset(eps_b, EPS)
            one_eps_b = singles.tile([P, 1], f32)
            nc.vector.memset(one_eps_b, 1.0 + EPS)

            # Each instruction should depend on at most ONE foreign
            # semaphore that is not yet known-satisfied; tiny "touch" ops
            # consume extra waits so the real consumers inherit them via
            # engine program order / already-observed clocks.
            touch_d = singles.tile([P, 1], f32)
            atouch_d = singles.tile([P, 1], f32)
            nc.scalar.activation(
                out=atouch_d, in_=w_cols[:, 0:1], func=ACTF.Copy
            )  # waits w_cols DMA on ACT
            nc.scalar.activation(
                out=atouch_d, in_=eps_b, func=ACTF.Copy
            )  # waits DVE memsets on ACT
            nc.scalar.activation(out=atouch_d, in_=one_eps_b, func=ACTF.Copy)

            # Pipeline shape (steady state, DMA-paced at ~12.7us/tile):
            #   DMA batch order g,x,s,o,q matches DVE op order, so DVE is
            #   never input-starved. The a/b accumulation ops LAG ONE TILE:
            #   a(t-1)/b(t-1) consume Ln outputs finished a full tile ago,
            #   so no DVE op ever waits on an ACT op of its own tile -- the
            #   cross-engine chains that serialized earlier versions
            #   (~15.5us/tile) are gone. DVE (~11.3us/tile) rides just
            #   behind the free-running 414 GB/s DMA stream.
            # The LAST tile's q is loaded in two halves and its Ln/accum
            # chain runs half-width, shrinking the post-stream drain from
            # ~6.5us to ~4.5us.
            d2 = d // 2
            prev = None
            for t in range(nt):
                # All big loads on the single qSP HWDGE ring: it alone runs
                # the 16 SDMA engines at ~100% line rate (~400 GB/s).
                # Splitting across SWDGE (or any second queue) makes the
                # engines interleave queues at packet granularity and COSTS
                # ~28% extra engine time per byte (measured).
                g = tgt_p.tile([P, d], f32)
                nc.sync.dma_start(out=g, in_=tgt_t[t])
                x = in_p.tile([P, d], f32)
                nc.sync.dma_start(out=x, in_=inp_t[t])
                s = st_p.tile([P, d], f32)
                nc.sync.dma_start(out=s, in_=st_t[t])
                o = ob_p.tile([P, d], f32)
                nc.sync.dma_start(out=o, in_=ob_t[t])
                q = tp_p.tile([P, d], f32)
                if t < nt - 1:
                    nc.sync.dma_start(out=q, in_=tp_t[t])
                else:
                    nc.sync.dma_start(out=q[:, 0:d2], in_=tp_t[t][:, 0:d2])
                    nc.sync.dma_start(out=q[:, d2:d], in_=tp_t[t][:, d2:d])

                # ---- wmse: diff on DVE, then Square(w*diff)+accum on ACT
                nc.vector.tensor_copy(touch_d, g[:, 0:1])  # consume g-DMA wait
                nc.vector.tensor_sub(g, g, x)  # g <- diff = target - input

                # ---- lagged accumulation of the previous tile's products
                if prev is not None:
                    pl1, pl2, pu, pv, pt = prev
                    nc.vector.scalar_tensor_tensor(
                        out=pl1,  # sink, in place
                        in0=pl1,
                        scalar=0.0,
                        in1=pu,
                        op0=ALU.bypass,
                        op1=ALU.mult,
                        accum_out=cl_cols[:, pt : pt + 1],
                    )
                    nc.vector.scalar_tensor_tensor(
                        out=pl2,  # sink, in place
                        in0=pl2,
                        scalar=0.0,
                        in1=pv,
                        op0=ALU.bypass,
                        op1=ALU.mult,
                        accum_out=cl_cols[:, ca + pt : ca + pt + 1],
                    )

                # ---- wcl inputs: u = st*ob; v' = (st-1)*ob (both pre-Ln)
                nc.vector.tensor_copy(touch_d, s[:, 0:1])  # consume s-DMA wait
                u = u_p.tile([P, d], f32)
                nc.vector.tensor_mul(u, s, o)
                v = v_p.tile([P, d], f32)
                nc.vector.scalar_tensor_tensor(
                    out=v,  # v <- v' = (st - 1) * ob
                    in0=s,
                    scalar=1.0,
                    in1=o,
                    op0=ALU.subtract,
                    op1=ALU.mult,
                )

                # ---- logs (ACT); bias/scale fold the affine into the LUT
                l1 = l1_p.tile([P, d], f32)
                l2 = l2_p.tile([P, d], f32)
                if t < nt - 1:
                    nc.scalar.activation(
                        out=l1, in_=q, func=ACTF.Ln, bias=eps_b, scale=1.0
                    )
                    nc.scalar.activation(
                        out=l2, in_=q, func=ACTF.Ln, bias=one_eps_b, scale=-1.0
                    )
                else:
                    for h0, h1 in ((0, d2), (d2, d)):
                        nc.scalar.activation(
                            out=l1[:, h0:h1],
                            in_=q[:, h0:h1],
                            func=ACTF.Ln,
                            bias=eps_b,
                            scale=1.0,
                        )
                        nc.scalar.activation(
                            out=l2[:, h0:h1],
                            in_=q[:, h0:h1],
                            func=ACTF.Ln,
                            bias=one_eps_b,
                            scale=-1.0,
                        )
                # Square AFTER the Lns in ACT program order: it feeds only
                # the mse store, so it must not sit between q landing and
                # the Ln->a/b tail chain.
                nc.scalar.activation(
                    out=x,  # sink; x is dead after the sub
                    in_=g,
                    func=ACTF.Square,
                    bias=0.0,
                    scale=w_cols[:, t : t + 1],
                    accum_out=mse_cols[:, t : t + 1],
                )
                prev = (l1, l2, u, v, t)

            # ---- final tile's lagged accums, half-width to shorten drain
            pl1, pl2, pu, pv, pt = prev
            for i, (h0, h1) in enumerate(((0, d2), (d2, d))):
                nc.vector.scalar_tensor_tensor(
                    out=pl1[:, h0:h1],
                    in0=pl1[:, h0:h1],
                    scalar=0.0,
                    in1=pu[:, h0:h1],
                    op0=ALU.bypass,
                    op1=ALU.mult,
                    accum_out=cl_cols[:, pt + i : pt + i + 1],
                )
                nc.vector.scalar_tensor_tensor(
                    out=pl2[:, h0:h1],
                    in0=pl2[:, h0:h1],
                    scalar=0.0,
                    in1=pv[:, h0:h1],
                    op0=ALU.bypass,
                    op1=ALU.mult,
                    accum_out=cl_cols[:, ca + pt + i : ca + pt + i + 1],
                )

            nc.sync.dma_start(out=out_mse[:, :], in_=mse_cols)
            nc.sync.dma_start(out=out_cl[:, :], in_=cl_cols)
    return nc


def _get_nc():
    bufs = int(os.environ.get("BASS_BUFS", "3"))
    if bufs not in _CACHE:
        nc = build(bufs=bufs)
        nc.finalize()  # runs Bacc's passes (event-sem wait splitting, regalloc)
        _CACHE[bufs] = nc
    return _CACHE[bufs]


def _install_profile_hook():
    """Register the NTFF profile hook that this container's stripped antenv
    lacks: a ctypes bridge into libaxon_pjrt.so (same ABI trn_boot.py uses).
    Only needed for trace=True runs."""
    if "antenv.axon_hooks" in sys.modules:
        return
    import contextlib
    import ctypes
    import types

    so_path = "/opt/axon/libaxon_pjrt.so"
    lib = ctypes.CDLL(so_path)
    if not hasattr(lib, "axon_start_nrt_profile"):
        return
    lib.axon_start_nrt_profile.argtypes = [
        ctypes.POINTER(ctypes.c_int64),
        ctypes.c_size_t,
    ]
    lib.axon_start_nrt_profile.restype = ctypes.c_int64
    lib.axon_stop_nrt_profile.argtypes = [ctypes.c_char_p]
    lib.axon_stop_nrt_profile.restype = ctypes.c_int64

    @contextlib.contextmanager
    def _hook(output_dir, device_ids):
        import jax

        jax.devices()
        if device_ids:
            ids = (ctypes.c_int64 * len(device_ids))(*device_ids)
            rc = lib.axon_start_nrt_profile(ids, len(device_ids))
        else:
            rc = lib.axon_start_nrt_profile(None, 0)
        if rc != 0:
            raise RuntimeError(f"axon_start_nrt_profile rc={rc}")
        try:
            yield
        finally:
            n = lib.axon_stop_nrt_profile(str(output_dir).encode())
            print(f"profile: {n} file(s) written to {output_dir}")

    mod = types.ModuleType("antenv.axon_hooks")
    mod.get_axon_ntff_profile_hook = lambda: _hook
    sys.modules["antenv.axon_hooks"] = mod


def kernel(**inputs):
    from concourse.bass_utils import run_bass_kernel_spmd

    nc = _get_nc()
    names = ["input", "target", "weight", "sub_target", "target_pre", "sub_obrT"]
    arrs = {k: np.ascontiguousarray(np.asarray(inputs[k], dtype=np.float32)) for k in names}
    in_maps = []
    for c in range(NCORES):
        sl = slice(c * ROWS, (c + 1) * ROWS)
        in_maps.append({k: np.ascontiguousarray(v[sl]) for k, v in arrs.items()})

    trace = os.environ.get("BASS_KERNEL_PROFILE", "0") == "1"
    if trace:
        _install_profile_hook()
    res = run_bass_kernel_spmd(nc, in_maps, list(range(NCORES)), trace=trace)

    ca = ROWS // P + 1
    mse_sum = 0.0
    cla_sum = 0.0
    clb_sum = 0.0
    for r in res.results:
        mse_sum += np.asarray(r["mse_partials"], dtype=np.float64).sum()
        cl = np.asarray(r["cl_partials"], dtype=np.float64)
        cla_sum += cl[:, :ca].sum()
        clb_sum += cl[:, ca:].sum()
    tot = float(N) * float(D)
    if trace and res.exec_time_ns is not None:
        print(f"HW exec time: {res.exec_time_ns} ns")
    return (
        np.asarray(np.float32(mse_sum / tot)),
        np.asarray(np.float32((clb_sum - cla_sum) / tot)),
    )
